# revision 1
# baseline (speedup 1.0000x reference)
"""Trainium2 Bass kernel for nn_Conv_39333310497378 (nms_detection).

Reference computation:
  x [16384, 1, 41, 40] f32, W [9, 50, 1, 6, 40] f32
  9 overlapping height-sections of x (section i = rows 4i..4i+8), each conv'd
  with its own [50, 1, 6, 40] kernel (VALID) -> [B, 50, 4, 1], max-pooled over
  the 4 -> [B, 50, 1, 1]; concat sections -> pots [B, 50, 9, 1];
  spks = (pots > 6.2) as 1.0/0.0.

Strategy (pure data parallelism over batch, 8 cores x 2048 samples):
  All 36 conv outputs j (= 4*sec + h) are dot-products of 240 consecutive
  elements of the flattened per-sample x row-block (elements 40j .. 40j+239)
  with per-(j, out-channel) weights.  Per core, x is staged host-side as a
  transposed [1664, 2048] bf16 array (flattened element-index major, batch
  minor, padded 1640 -> 13*128).  Each 128-element chunk c becomes a matmul
  stationary operand [128, 128-batch-tile]; a host-precomputed banded weight
  tile Wb[c] [128, <=450] (moving operand) scatters the chunk's contribution
  into PSUM columns 50j+o for every j whose input window overlaps the chunk.
  PSUM accumulates across the 13 chunks (per-element has_written semantics;
  start=True on the first matmul touching each 512-col bank).  VectorE then
  max-reduces h (groups of 4 columns-of-50), writes pots in [o, sec] layout,
  thresholds for spks, and both DMA out contiguously.
"""
import math
import sys

import numpy as np

sys.path.insert(0, "/opt/trn_rl_repo")

import ml_dtypes  # noqa: E402

import concourse.bass as bass  # noqa: E402
import concourse.mybir as mybir  # noqa: E402
import concourse.tile as tile  # noqa: E402
from concourse import bacc  # noqa: E402
from concourse.bass_utils import run_bass_kernel_spmd  # noqa: E402

BF16 = mybir.dt.bfloat16
F32 = mybir.dt.float32

B, ROWS, WIDTH = 16384, 41, 40
NSEC, OC = 9, 50
NJ = 36
THRESHOLD = 6.2
NCORES = 8
BC = B // NCORES            # 2048 samples per core
E = ROWS * WIDTH            # 1640 elements per sample
NCHUNK = 13
EP = NCHUNK * 128           # 1664 (padded)
BT = 128                    # batch tile = psum partition dim
GRP = 4                     # batch tiles per DMA group
PSUM_COLS = 2048            # 4 banks


def _windows():
    jlo, jhi = [], []
    for c in range(NCHUNK):
        js = [j for j in range(NJ)
              if 40 * j < 128 * c + 128 and 40 * j + 240 > 128 * c]
        jlo.append(min(js)); jhi.append(max(js))
    return jlo, jhi


def _segments(jlo, jhi):
    """Matmul segments in emission order: (chunk, col_a, col_b, start, stop).

    PSUM accumulate flags are only per-element in principle; both CoreSim and
    the safe HW model require each matmul to be wholly first-write (pending
    zero) or wholly accumulate within its bank.  Chunk col-windows have
    nondecreasing ends, so each chunk/bank intersection splits into a "fresh"
    piece (cols beyond everything written so far in the bank) and an
    "accumulate" piece.  Only the very first matmul of a bank carries
    start=True (it marks the entire bank pending-zero).
    """
    nbanks = math.ceil(NJ * OC / 512)
    prev_hi = [512 * k for k in range(nbanks)]
    bank_started = [False] * nbanks
    pieces = []
    for c in range(NCHUNK):
        A, Bc = jlo[c] * OC, (jhi[c] + 1) * OC
        for k in range(nbanks):
            lo, hi = max(A, 512 * k), min(Bc, 512 * (k + 1))
            if lo >= hi:
                continue
            old_hi = prev_hi[k]
            assert lo <= old_hi, f"coverage gap in bank {k}: {lo} > {old_hi}"
            if hi > old_hi:                       # fresh columns
                pieces.append([c, old_hi, hi, not bank_started[k], False])
                bank_started[k] = True
                prev_hi[k] = hi
            if lo < min(hi, old_hi):              # accumulate columns
                pieces.append([c, lo, min(hi, old_hi), False, False])
    last = {}
    for idx, p in enumerate(pieces):
        last[p[1] // 512] = idx
    for idx in last.values():
        pieces[idx][4] = True
    return [tuple(p) for p in pieces]


def _build_wband(W, jlo, jhi):
    """[NCHUNK, 128, 450] f32 banded weights; col (j-jlo)*50+o, row = elem-128c."""
    Wsq = np.asarray(W, np.float32)[:, :, 0]          # [9, 50, 6, 40]
    Wb = np.zeros((NCHUNK, 128, 450), np.float32)
    for c in range(NCHUNK):
        for j in range(jlo[c], jhi[c] + 1):
            sec = j // 4
            e0, e1 = max(40 * j, 128 * c), min(40 * j + 240, 128 * (c + 1))
            es = np.arange(e0, e1)
            Wb[c, es - 128 * c, (j - jlo[c]) * OC:(j - jlo[c] + 1) * OC] = \
                Wsq[sec][:, es // 40 - j, es % 40].T
    return Wb


def _build_program(bc=BC):
    """One-core SPMD program operating on a [EP, bc] transposed x shard."""
    jlo, jhi = _windows()
    segs = _segments(jlo, jhi)
    n_bt = bc // BT
    n_grp = max(1, n_bt // GRP)
    grp = n_bt // n_grp

    ob = 2 if n_bt % 2 == 0 else 1          # batch tiles per output DMA

    nc = bacc.Bacc(None)
    xT_d = nc.dram_tensor("xT", [NCHUNK, 128, bc], BF16, kind="ExternalInput")
    wb_d = nc.dram_tensor("Wb", [128, NCHUNK, 450], BF16, kind="ExternalInput")
    pots_d = nc.dram_tensor("pots", [n_bt, BT, OC * NSEC], BF16,
                            kind="ExternalOutput")
    spks_d = nc.dram_tensor("spks", [n_bt, BT, OC * NSEC], BF16,
                            kind="ExternalOutput")

    with tile.TileContext(nc) as tc:
        with (
            tc.tile_pool(name="w", bufs=1) as wpool,
            tc.tile_pool(name="x", bufs=3) as xpool,
            tc.tile_pool(name="out", bufs=2) as opool,
            tc.tile_pool(name="ps", bufs=2, space="PSUM") as pspool,
        ):
            # weights go on the ACT HWDGE ring so the sync ring starts on x
            wtile = wpool.tile([128, NCHUNK, 450], BF16)
            nc.scalar.dma_start(wtile[:], wb_d[:])
            nthr = wpool.tile([128, 1], F32, tag="nthr")
            nc.any.memset(nthr[:], -THRESHOLD)
            po = sp2 = None
            x0 = None
            for g in range(n_grp):
                if g == 0:
                    # group 0 loads per-chunk so PE can start on chunk 0
                    # while the rest is still in flight
                    x0 = [xpool.tile([128, grp * BT], BF16, tag=f"x0_{c}",
                                     name=f"x0_{c}")
                          for c in range(NCHUNK)]
                    for c in range(NCHUNK):
                        nc.sync.dma_start(x0[c][:], xT_d[c, :, 0:grp * BT])
                else:
                    # one fused DMA for all 13 chunks of this batch group
                    xg = xpool.tile([128, NCHUNK, grp * BT], BF16, tag="xg")
                    nc.sync.dma_start(
                        xg[:],
                        xT_d[:, :, g * grp * BT:(g + 1) * grp * BT]
                        .rearrange("c p b -> p c b"))
                for tl in range(grp):
                    bt = g * grp + tl
                    s = bt % ob
                    if s == 0:
                        po = opool.tile([128, ob, OC * NSEC], BF16, tag="po")
                        sp2 = opool.tile([128, ob, OC * NSEC], BF16, tag="sp")
                    ps = pspool.tile([128, PSUM_COLS], F32, tag="ps")
                    for (c, a, b, st, stp) in segs:
                        lhsT = (x0[c][:, tl * BT:(tl + 1) * BT] if g == 0
                                else xg[:, c, tl * BT:(tl + 1) * BT])
                        nc.tensor.matmul(
                            ps[:, a:b], lhsT,
                            wtile[:, c, a - jlo[c] * OC: b - jlo[c] * OC],
                            start=st, stop=stp)
                    # [p, i, o, h] view, h innermost -> one reduce_max over X
                    psv = ps[:, :NJ * OC].rearrange(
                        "p (i h o) -> p i o h", h=4, o=OC)
                    pov = po[:, s, :].rearrange("p (o i) -> p i o", i=NSEC)
                    nc.vector.tensor_reduce(
                        pov, psv, axis=mybir.AxisListType.X,
                        op=mybir.AluOpType.max)
                    # spks = Relu(Sign(pots - thr)) on the otherwise-idle ACT
                    nc.scalar.activation(
                        sp2[:, s, :], po[:, s, :],
                        mybir.ActivationFunctionType.Sign, bias=nthr[:])
                    nc.scalar.activation(
                        sp2[:, s, :], sp2[:, s, :],
                        mybir.ActivationFunctionType.Relu)
                    if s == ob - 1:
                        # stores go out on the ACT HWDGE ring so input
                        # prefetch never queues behind them on the sync ring
                        t0 = bt - (ob - 1)
                        nc.scalar.dma_start(
                            pots_d[t0:t0 + ob].rearrange("t p n -> p t n"),
                            po[:])
                        nc.scalar.dma_start(
                            spks_d[t0:t0 + ob].rearrange("t p n -> p t n"),
                            sp2[:])
    nc.compile()
    return nc


_PROGRAM_CACHE = {}


def _get_program(bc=BC):
    if bc not in _PROGRAM_CACHE:
        _PROGRAM_CACHE[bc] = _build_program(bc)
    return _PROGRAM_CACHE[bc]


def _prep_inputs(x, W):
    jlo, jhi = _windows()
    wb = _build_wband(W, jlo, jhi).transpose(1, 0, 2)       # [128, 13, 450]
    wb = np.ascontiguousarray(wb).astype(ml_dtypes.bfloat16)
    xf = np.asarray(x, np.float32).reshape(B, E)
    in_maps = []
    for ci in range(NCORES):
        xs = xf[ci * BC:(ci + 1) * BC]
        xpad = np.zeros((BC, EP), np.float32)
        xpad[:, :E] = xs
        xT = np.ascontiguousarray(xpad.T).astype(ml_dtypes.bfloat16)
        in_maps.append({"xT": xT.reshape(NCHUNK, 128, BC), "Wb": wb})
    return in_maps


def kernel(x, W):
    nc = _get_program()
    in_maps = _prep_inputs(x, W)
    res = run_bass_kernel_spmd(nc, in_maps, list(range(NCORES)))
    pots = np.concatenate(
        [np.asarray(r["pots"]).astype(np.float32).reshape(BC, OC * NSEC)
         for r in res.results], axis=0)
    spks = np.concatenate(
        [np.asarray(r["spks"]).astype(np.float32).reshape(BC, OC * NSEC)
         for r in res.results], axis=0)
    pots = pots.reshape(B, OC, NSEC, 1)
    spks = spks.reshape(B, OC, NSEC, 1)
    return pots, spks



# revision 3
# speedup vs baseline: 1.1873x; 1.1873x over previous
"""Trainium2 Bass kernel for nn_Conv_39333310497378 (nms_detection).

Reference computation:
  x [16384, 1, 41, 40] f32, W [9, 50, 1, 6, 40] f32
  36 sliding 6-row windows j (window j = rows j..j+5, section sec=j//4),
  out[b, j, o] = <x[b, rows j..j+5, :], W[sec, o]>  (240-elem dot)
  pots[b, sec, o] = max over h=j%4 of out[b, 4 sec+h, o]
  spks = (pots > 6.2) as 1.0/0.0.

Strategy (data parallel over batch, 8 cores x 2048 samples):
  Per batch tile of 128 samples the 36x50 output columns accumulate in
  PSUM (cols j*50+o, 1800 of 2048 across 4 banks).  x is chunked into 14
  three-row chunks of 120 elements (stride 120); each window is covered
  by 2-3 chunks (96 window-chunk pieces = 4800 moving columns per tile,
  vs 5850 for 128-aligned chunks).  Chunk c's x slice [120, 128] is the
  matmul stationary operand (fp8e3m4: 4x fast-weight-load), the banded
  per-chunk weights [120, <=400] (fp8e3m4) the moving operand.  One
  matmul per (chunk x psum bank); the first matmul touching a bank
  carries start=True, which clears the whole bank's has_written bits, so
  later matmuls overwrite-or-accumulate per element (no fresh/accumulate
  splitting).  VectorE max-pools h=4 via a two-level tensor_tensor max
  tree (PSUM pair-max -> bf16 SBUF pair-max), GpSimdE computes spks with
  a single is_gt, and outputs stream out per 4-tile group as contiguous
  [128, 4, 450] bf16 blocks on the scalar ring.
"""
import sys

import numpy as np

sys.path.insert(0, "/opt/trn_rl_repo")

import ml_dtypes  # noqa: E402

import concourse.bass as bass  # noqa: E402
import concourse.mybir as mybir  # noqa: E402
import concourse.tile as tile  # noqa: E402
from concourse import bacc  # noqa: E402
from concourse.bass_utils import run_bass_kernel_spmd  # noqa: E402

FP8 = mybir.dt.float8e3
BF16 = mybir.dt.bfloat16
F32 = mybir.dt.float32
NP_FP8 = ml_dtypes.float8_e3m4

B, ROWS, WIDTH = 16384, 41, 40
NSEC, OC, NJ = 9, 50, 36
THRESHOLD = 6.2
NCORES = 8
BC = B // NCORES            # 2048 samples per core
E = ROWS * WIDTH            # 1640 elements per sample
BT = 128                    # batch tile = psum partition dim
NT = BC // BT               # 16 batch tiles per core
GRP = 4                     # batch tiles per input DMA group
NG = NT // GRP              # 4 input groups
OG = 4                      # batch tiles per output DMA group
WLEN = 240                  # window length (6 rows x 40)

CLEN = 120                  # chunk length (3 rows)
CSTART = [120 * m for m in range(14)]
NCHUNK = len(CSTART)
EP = CSTART[-1] + CLEN      # 1680 padded elements per sample


def _plan():
    """Greedy min-cover of each window by chunks.

    Returns (cov, cwin, pieces):
      cov[c]   = list of (j, e0, e1) element ranges chunk c contributes
      cwin[c]  = (A, B) psum column window of chunk c
      pieces   = [(c, A, lo, hi, start, stop)] matmuls in emission order
    """
    cov = [[] for _ in range(NCHUNK)]
    for j in range(NJ):
        lo, hi = 40 * j, 40 * j + WLEN
        pos = lo
        while pos < hi:
            cands = [c for c, s in enumerate(CSTART) if s <= pos < s + CLEN]
            assert cands, f"window {j} uncovered at {pos}"
            best = max(cands, key=lambda c: CSTART[c] + CLEN)
            e1 = min(CSTART[best] + CLEN, hi)
            cov[best].append((j, pos, e1))
            pos = e1
    cwin = []
    for c in range(NCHUNK):
        js = [j for j, _, _ in cov[c]]
        assert js, f"chunk {c} unused"
        assert js == sorted(js) and js[-1] - js[0] == len(js) - 1, \
            f"chunk {c} windows not contiguous: {js}"
        cwin.append((OC * js[0], OC * (js[-1] + 1)))
    pieces = []
    seen = set()
    last = {}
    for c in range(NCHUNK):
        A, Bc = cwin[c]
        for k in range(A // 512, (Bc - 1) // 512 + 1):
            lo, hi = max(A, 512 * k), min(Bc, 512 * (k + 1))
            if lo >= hi:
                continue
            st = k not in seen
            if st:
                assert lo == 512 * k, f"bank {k} first piece lo={lo}"
                seen.add(k)
            pieces.append([c, A, lo, hi, st, False])
            last[k] = len(pieces) - 1
    for idx in last.values():
        pieces[idx][5] = True
    return cov, cwin, [tuple(p) for p in pieces]


def _build_wband(W):
    """Per-chunk banded weight tiles, concatenated -> ([120, TOTW], offsets)."""
    cov, cwin, _ = _plan()
    Wsq = np.asarray(W, np.float32)[:, :, 0]          # [9, 50, 6, 40]
    tiles, offs, off = [], [], 0
    for c in range(NCHUNK):
        A, Bc = cwin[c]
        wt = np.zeros((CLEN, Bc - A), np.float32)
        for (j, e0, e1) in cov[c]:
            es = np.arange(e0, e1)
            wt[es - CSTART[c], OC * j - A:OC * (j + 1) - A] = \
                Wsq[j // 4][:, es // 40 - j, es % 40].T
        tiles.append(wt)
        offs.append(off)
        off += Bc - A
    return np.concatenate(tiles, axis=1), offs


def _build_program(bc=BC):
    """One-core SPMD program for a [CLEN, NG, NCHUNK, GRP*BT] fp8 x shard."""
    _, cwin, pieces = _plan()
    totw = sum(b - a for a, b in cwin)
    woff = np.cumsum([0] + [b - a for a, b in cwin]).tolist()

    nc = bacc.Bacc(None)
    xT_d = nc.dram_tensor("xT", [CLEN, NG, NCHUNK, GRP * BT], FP8,
                          kind="ExternalInput")
    wb_d = nc.dram_tensor("Wb", [CLEN, totw], FP8, kind="ExternalInput")
    pots_d = nc.dram_tensor("pots", [NT // OG, BT, OG, OC * NSEC], BF16,
                            kind="ExternalOutput")
    spks_d = nc.dram_tensor("spks", [NT // OG, BT, OG, OC * NSEC], BF16,
                            kind="ExternalOutput")

    # group-0 x arrives in 4 sub-tiles so matmuls can start on the first
    # chunks while the rest is still in flight
    g0split = [(0, 4), (4, 4), (8, 4), (12, 2)]

    with tile.TileContext(nc) as tc:
        with (
            tc.tile_pool(name="w", bufs=1) as wpool,
            tc.tile_pool(name="x", bufs=2) as xpool,
            tc.tile_pool(name="t", bufs=2) as tpool,
            tc.tile_pool(name="out", bufs=2) as opool,
            tc.tile_pool(name="ps", bufs=2, space="PSUM") as pspool,
        ):
            wtile = wpool.tile([CLEN, totw], FP8)
            nc.scalar.dma_start(wtile[:], wb_d[:])
            x0 = []
            for i, (c0, nch) in enumerate(g0split):
                t = wpool.tile([CLEN, nch, GRP * BT], FP8, tag=f"x0_{i}",
                               name=f"x0_{i}")
                nc.sync.dma_start(t[:], xT_d[:, 0, c0:c0 + nch, :])
                x0.append(t)

            po = sp = None
            xg = None
            for g in range(NG):
                if g > 0:
                    xg = xpool.tile([CLEN, NCHUNK, GRP * BT], FP8, tag="xg")
                    nc.sync.dma_start(xg[:], xT_d[:, g])
                for tl in range(GRP):
                    bt = g * GRP + tl
                    s = bt % OG
                    if s == 0:
                        po = opool.tile([BT, OG, OC * NSEC], BF16, tag="po")
                        sp = opool.tile([BT, OG, OC * NSEC], BF16, tag="sp")
                    ps = pspool.tile([BT, 2048], F32, tag="ps")
                    cp = tpool.tile([BT, NJ * OC], BF16, tag="cp")
                    t2 = tpool.tile([BT, 2 * OC * NSEC], BF16, tag="t2")
                    for (c, A, lo, hi, st, stp) in pieces:
                        if g == 0:
                            gi = 0 if c < 4 else 1 if c < 8 else 2 if c < 12 else 3
                            lhsT = x0[gi][:, c - g0split[gi][0],
                                          tl * BT:(tl + 1) * BT]
                        else:
                            lhsT = xg[:, c, tl * BT:(tl + 1) * BT]
                        nc.tensor.matmul(
                            ps[:, lo:hi], lhsT,
                            wtile[:, woff[c] + lo - A:woff[c] + hi - A],
                            start=st, stop=stp, skip_group_check=True)
                    # ScalarE: PSUM (i,h,o) -> SBUF bf16 h-major (h,i,o),
                    # then the h=4 max tree + threshold run as contiguous
                    # bf16 SBUF ops on VectorE (DVE has 1 PSUM read port, so
                    # tensor_tensor straight from PSUM is illegal).
                    inv = ps[:, :NJ * OC].rearrange("p (i h o) -> p h i o",
                                                    h=4, o=OC)
                    outv = cp[:].rearrange("p (h i o) -> p h i o",
                                           i=NSEC, o=OC)
                    nc.scalar.activation(outv, inv,
                                         mybir.ActivationFunctionType.Copy)
                    nc.vector.tensor_max(t2[:], cp[:, 0:900], cp[:, 900:1800])
                    nc.vector.tensor_max(po[:, s, :], t2[:, 0:450],
                                         t2[:, 450:900])
                    nc.vector.tensor_scalar(
                        sp[:, s, :], po[:, s, :], float(THRESHOLD), None,
                        mybir.AluOpType.is_gt)
                    if s == OG - 1:
                        gi = bt // OG
                        nc.scalar.dma_start(pots_d[gi], po[:])
                        nc.scalar.dma_start(spks_d[gi], sp[:])
    nc.compile()
    return nc


_PROGRAM_CACHE = {}


def _get_program(bc=BC):
    if bc not in _PROGRAM_CACHE:
        _PROGRAM_CACHE[bc] = _build_program(bc)
    return _PROGRAM_CACHE[bc]


def _prep_inputs(x, W):
    wb, _ = _build_wband(W)
    wb8 = np.ascontiguousarray(wb).astype(NP_FP8)
    xf = np.asarray(x, np.float32).reshape(B, E)
    in_maps = []
    for ci in range(NCORES):
        xpad = np.zeros((BC, EP), np.float32)
        xpad[:, :E] = xf[ci * BC:(ci + 1) * BC]
        # [bc, EP] -> [NG, GRP*BT, NCHUNK, CLEN] -> [CLEN, NG, NCHUNK, GRP*BT]
        x4 = xpad.reshape(NG, GRP * BT, NCHUNK, CLEN).transpose(3, 0, 2, 1)
        in_maps.append({"xT": np.ascontiguousarray(x4).astype(NP_FP8),
                        "Wb": wb8})
    return in_maps


def kernel(x, W):
    nc = _get_program()
    in_maps = _prep_inputs(x, W)
    res = run_bass_kernel_spmd(nc, in_maps, list(range(NCORES)))
    pots_l, spks_l = [], []
    for r in res.results:
        # [NT//OG, BT, OG, 450] -> [NT//OG, OG, BT, 450] -> [BC, 9, 50]
        p4 = np.asarray(r["pots"]).astype(np.float32)
        s4 = np.asarray(r["spks"]).astype(np.float32)
        pots_l.append(p4.transpose(0, 2, 1, 3).reshape(BC, NSEC, OC))
        spks_l.append(s4.transpose(0, 2, 1, 3).reshape(BC, NSEC, OC))
    pots = np.concatenate(pots_l, axis=0).transpose(0, 2, 1).copy()
    spks = np.concatenate(spks_l, axis=0).transpose(0, 2, 1).copy()
    return pots.reshape(B, OC, NSEC, 1), spks.reshape(B, OC, NSEC, 1)
